# revision 3
# baseline (speedup 1.0000x reference)
"""Trainium2 Bass kernel for a 3-net MLP + masked mean-pooled cross-attention.

For each batch segment i (B=32 segments data-parallel across 8 NeuronCores):
    q/k/v = MLP3(x) per token (LeakyReLU; eval-BatchNorm folded into the
    second matmul's weights host-side), then
    emb_a[i] = mean over valid a-rows of softmax(qa kb^T / 32, key-masked) @ vb
    emb_b[i] = symmetric.

Key algebraic points exploited:
  * The mean over query rows commutes with the attention value matmul:
    emb = (sum_q w_q softmax_row_q) @ V = u @ V with u a [Lk] vector, so the
    big [Lq, D] attention-output matmul is never formed.
  * BatchNorm (eval mode) is affine -> folded into W2/b2 host-side.
  * The 1/32 score scale is folded into the q-net weights host-side.
  * Key-side masking is a rank-1 additive update (ones (x) mask-row) applied
    by one K=1 matmul into the score PSUM accumulation group; exp then
    underflows masked entries to exactly 0.
All matmul operands are bf16 with fp32 PSUM accumulation.
"""

import os
import sys

import numpy as np

for _p in ("/opt/trn_rl_repo", "/root/.axon_site/_ro/trn_rl_repo"):
    if os.path.isdir(_p) and _p not in sys.path:
        sys.path.insert(0, _p)

import ml_dtypes  # noqa: E402

B, LA, LB, D, H, P = 32, 1024, 1024, 1024, 256, 3
BN_EPS = 1e-5
SCALE = 32.0
N_CORES = 8
SEG = B // N_CORES  # segments per core
TOKBLK = 512
NEG = -1e6  # additive key mask; exp(x + NEG) underflows to exactly 0 in fp32
RAGGED = True  # specialize score loops on 128-padded lengths (host-baked)

_CACHE = {}
LAST_RESULTS = None


def _round_up(x, m):
    return (x + m - 1) // m * m


def _chunks(kpad):
    """Split [0, kpad) into free-dim chunks of <=512 (PSUM bank limit)."""
    out = []
    c = 0
    while c < kpad:
        w = min(512, kpad - c)
        out.append((c, w))
        c += w
    return out


def _build_program(sched):
    """sched[(dirn, pos)] = (n_qtiles, kpad): per segment-position loop
    structure, shared by all cores (SPMD). dirn 0: q from side a, k/v from b."""
    import concourse.bacc as bacc
    import concourse.mybir as mybir
    import concourse.tile as tile

    F32 = mybir.dt.float32
    BF16 = mybir.dt.bfloat16
    AF = mybir.ActivationFunctionType
    ALU = mybir.AluOpType
    AX = mybir.AxisListType

    nc = bacc.Bacc(
        "TRN2",
        target_bir_lowering=False,
        debug=False,
        enable_asserts=False,
        num_devices=N_CORES,
    )

    xa_d = nc.dram_tensor("xa", [SEG * LA, D], BF16, kind="ExternalInput").ap()
    xb_d = nc.dram_tensor("xb", [SEG * LB, D], BF16, kind="ExternalInput").ap()
    w1_d = nc.dram_tensor("w1", [P, D, H], BF16, kind="ExternalInput").ap()
    w2_d = nc.dram_tensor("w2", [P, H, D], BF16, kind="ExternalInput").ap()
    b1_d = nc.dram_tensor("b1", [P, H], F32, kind="ExternalInput").ap()
    b2_d = nc.dram_tensor("b2", [P, D], F32, kind="ExternalInput").ap()
    km_d = nc.dram_tensor("km", [2, SEG, LA], BF16, kind="ExternalInput").ap()
    wb_d = nc.dram_tensor("wb", [2, SEG, LA], F32, kind="ExternalInput").ap()
    o_d = nc.dram_tensor("o", [2, SEG, D], F32, kind="ExternalOutput").ap()

    DT = D // 128  # 8 d-tiles
    HT = H // 128  # 2 h-tiles
    NBLK = LA // TOKBLK  # token blocks per side

    with tile.TileContext(nc) as tc:
        with (
            tc.tile_pool(name="consts", bufs=1) as consts,
            tc.tile_pool(name="qkv", bufs=1) as qkvp,
            tc.tile_pool(name="xt", bufs=2) as xtp,
            tc.tile_pool(name="ypool", bufs=3) as ypool,
            tc.tile_pool(name="hbn", bufs=2) as hbnp,
            tc.tile_pool(name="epool", bufs=3) as epool,
            tc.tile_pool(name="stats", bufs=8) as stats,
            tc.tile_pool(name="ubc", bufs=2) as ubcp,
            tc.tile_pool(name="scratch", bufs=2) as scrp,
            tc.tile_pool(name="embp", bufs=2) as embp,
            tc.tile_pool(name="psA", bufs=2, space="PSUM") as psA,
            tc.tile_pool(name="psS", bufs=4, space="PSUM") as psS,
            tc.tile_pool(name="psU", bufs=1, space="PSUM") as psU,
            tc.tile_pool(name="dramp", bufs=2, space="DRAM") as dramp,
        ):
            # ---- constants ----
            w1_sb = []
            for dt in range(DT):
                t = consts.tile([128, P * H], BF16, name=f"w1sb{dt}")
                nc.sync.dma_start(
                    out=t,
                    in_=w1_d[:, dt * 128 : (dt + 1) * 128, :].transpose([1, 0, 2]),
                )
                w1_sb.append(t)
            w2_sb = []
            for ht in range(HT):
                t = consts.tile([128, P * D], BF16, name=f"w2sb{ht}")
                nc.sync.dma_start(
                    out=t,
                    in_=w2_d[:, ht * 128 : (ht + 1) * 128, :].transpose([1, 0, 2]),
                )
                w2_sb.append(t)
            b1_sb = consts.tile([128, P * HT], F32)
            nc.sync.dma_start(out=b1_sb, in_=b1_d.rearrange("p (t h) -> h (p t)", h=128))
            b2_sb = consts.tile([128, P * DT], F32)
            nc.sync.dma_start(out=b2_sb, in_=b2_d.rearrange("p (t d) -> d (p t)", d=128))
            km_sb = consts.tile([1, 2 * SEG * LA], BF16)
            nc.sync.dma_start(out=km_sb, in_=km_d.rearrange("a s l -> (a s l)").unsqueeze(0))
            wb_sb = consts.tile([128, 2 * SEG * 8], F32)
            nc.sync.dma_start(out=wb_sb, in_=wb_d.rearrange("a s (t p) -> p (a s t)", p=128))
            ones_sb = consts.tile([1, 128], BF16)
            nc.vector.memset(ones_sb, 1.0)

            def mlp(seg, x2d, qkv):
                """Fill qkv[p][dt]: [128, L] bf16 tiles (feature-major, partition=d)."""
                for blk in range(NBLK):
                    tok0 = seg * LA + blk * TOKBLK
                    xt = xtp.tile([128, DT, TOKBLK], BF16, tag="xt", name=f"xt{seg}{blk}")
                    for dt in range(DT):
                        nc.sync.dma_start(
                            out=xt[:, dt, :],
                            in_=x2d[tok0 : tok0 + TOKBLK, dt * 128 : (dt + 1) * 128],
                            transpose=True,
                        )
                    hbn = {}
                    for p in range(P):
                        for ht in range(HT):
                            hp = psA.tile([128, TOKBLK], F32, tag="ps_mlp", name=f"hp{seg}{blk}{p}{ht}")
                            for dt in range(DT):
                                nc.tensor.matmul(
                                    hp,
                                    w1_sb[dt][:, p * H + ht * 128 : p * H + ht * 128 + 128],
                                    xt[:, dt, :],
                                    start=(dt == 0),
                                    stop=(dt == DT - 1),
                                )
                            y = ypool.tile([128, TOKBLK], F32, tag="y", name=f"y{seg}{blk}{p}{ht}")
                            nc.scalar.activation(
                                out=y, in_=hp, func=AF.Identity,
                                bias=b1_sb[:, p * HT + ht : p * HT + ht + 1],
                            )
                            hb = hbnp.tile([128, TOKBLK], BF16, tag=f"hbn{p}{ht}", name=f"hbn{seg}{blk}{p}{ht}")
                            # LeakyReLU: max(0.01*y, y)
                            nc.vector.scalar_tensor_tensor(
                                out=hb, in0=y, scalar=0.01, in1=y,
                                op0=ALU.mult, op1=ALU.max,
                            )
                            hbn[(p, ht)] = hb
                    for p in range(P):
                        for dt in range(DT):
                            op = psA.tile([128, TOKBLK], F32, tag="ps_mlp", name=f"op{seg}{blk}{p}{dt}")
                            for ht in range(HT):
                                nc.tensor.matmul(
                                    op,
                                    w2_sb[ht][:, p * D + dt * 128 : p * D + dt * 128 + 128],
                                    hbn[(p, ht)],
                                    start=(ht == 0),
                                    stop=(ht == HT - 1),
                                )
                            nc.scalar.activation(
                                out=qkv[p][dt][:, blk * TOKBLK : (blk + 1) * TOKBLK],
                                in_=op, func=AF.Identity,
                                bias=b2_sb[:, p * DT + dt : p * DT + dt + 1],
                            )

            def attention(seg, dirn, q_tiles, k_tiles, v_tiles):
                n_qt, kpad = sched[(dirn, seg)]
                kch = _chunks(kpad)
                bd = dirn * SEG + seg
                u_ps = psU.tile([1, kpad], F32, tag="ps_u", name=f"u{bd}")

                def softmax_u(qt, s_list):
                    # negm = -rowmax over all chunks
                    nm = []
                    for i, (c0, cw) in enumerate(kch):
                        t = stats.tile([128, 1], F32, tag="negm_c", name=f"negmc{bd}_{qt}_{i}")
                        nc.vector.reduce_max(out=t, in_=s_list[i][:, :cw], axis=AX.X, negate=True)
                        nm.append(t)
                    negm = nm[0]
                    for i in range(1, len(nm)):
                        t = stats.tile([128, 1], F32, tag="negm_t", name=f"negmt{bd}_{qt}_{i}")
                        nc.vector.tensor_tensor(out=t, in0=negm, in1=nm[i], op=ALU.min)
                        negm = t
                    e = epool.tile([128, kpad], BF16, tag="e", name=f"e{bd}_{qt}")
                    zs = []
                    for i, (c0, cw) in enumerate(kch):
                        z = stats.tile([128, 1], F32, tag="z_c", name=f"z{bd}_{qt}_{i}")
                        nc.scalar.activation(
                            out=e[:, c0 : c0 + cw], in_=s_list[i][:, :cw],
                            func=AF.Exp, bias=negm, scale=1.0, accum_out=z,
                        )
                        zs.append(z)
                    ztot = zs[0]
                    for i in range(1, len(zs)):
                        t = stats.tile([128, 1], F32, tag="z_t", name=f"zt{bd}_{qt}_{i}")
                        nc.vector.tensor_tensor(out=t, in0=ztot, in1=zs[i], op=ALU.add)
                        ztot = t
                    rz = stats.tile([128, 1], F32, tag="rz", name=f"rz{bd}_{qt}")
                    nc.vector.reciprocal(out=rz, in_=ztot)
                    w = stats.tile([128, 1], BF16, tag="w", name=f"w{bd}_{qt}")
                    nc.vector.tensor_tensor(
                        out=w, in0=wb_sb[:, bd * 8 + qt : bd * 8 + qt + 1], in1=rz,
                        op=ALU.mult,
                    )
                    for i, (c0, cw) in enumerate(kch):
                        nc.tensor.matmul(
                            u_ps[:, c0 : c0 + cw], w, e[:, c0 : c0 + cw],
                            start=(qt == 0), stop=(qt == n_qt - 1),
                        )

                pend = None  # softmax of qt issued after scores of qt+1 (PE keeps busy)
                for qt in range(n_qt):
                    s_list = []
                    for ci, (c0, cw) in enumerate(kch):
                        sp = psS.tile([128, 512], F32, tag="ps_s", name=f"s{bd}_{qt}_{ci}")
                        for dt in range(DT):
                            nc.tensor.matmul(
                                sp[:, :cw],
                                q_tiles[dt][:, qt * 128 : (qt + 1) * 128],
                                k_tiles[dt][:, c0 : c0 + cw],
                                start=(dt == 0),
                                stop=False,
                            )
                        nc.tensor.matmul(
                            sp[:, :cw],
                            ones_sb,
                            km_sb[:, bd * LA + c0 : bd * LA + c0 + cw],
                            start=False,
                            stop=True,
                        )
                        s_list.append(sp)
                    if pend is not None:
                        softmax_u(*pend)
                    pend = (qt, s_list)
                softmax_u(*pend)

                # u -> SBUF -> DRAM -> partition-broadcast; emb via DVE mul-reduce
                u_sb = stats.tile([1, kpad], BF16, tag="u_sb", name=f"usb{bd}")
                for c0, cw in kch:
                    nc.scalar.activation(out=u_sb[:, c0 : c0 + cw], in_=u_ps[:, c0 : c0 + cw], func=AF.Identity)
                u_dr = dramp.tile([1, kpad], BF16, tag="u_dr", name=f"udr{bd}")
                nc.sync.dma_start(out=u_dr, in_=u_sb)
                u_bc = ubcp.tile([128, kpad], BF16, tag="u_bc", name=f"ubc{bd}")
                nc.sync.dma_start(out=u_bc, in_=u_dr[0].partition_broadcast(128))
                emb_sb = embp.tile([128, DT], F32, tag="emb", name=f"emb{bd}")
                for dt in range(DT):
                    prod = scrp.tile([128, kpad], BF16, tag="prod", name=f"prod{bd}_{dt}")
                    # (v * 1.0) * u_bc elementwise; accum_out = row-sum = emb chunk
                    nc.vector.scalar_tensor_tensor(
                        out=prod, in0=v_tiles[dt][:, :kpad], scalar=1.0, in1=u_bc,
                        op0=ALU.mult, op1=ALU.mult,
                        accum_out=emb_sb[:, dt : dt + 1],
                    )
                nc.sync.dma_start(
                    out=o_d[dirn, seg].rearrange("(t p) -> p t", p=128), in_=emb_sb
                )

            for seg in range(SEG):
                qkv_a = [
                    [qkvp.tile([128, LA], BF16, tag=f"qkva{p}{dt}", name=f"qkva{seg}_{p}_{dt}") for dt in range(DT)]
                    for p in range(P)
                ]
                qkv_b = [
                    [qkvp.tile([128, LB], BF16, tag=f"qkvb{p}{dt}", name=f"qkvb{seg}_{p}_{dt}") for dt in range(DT)]
                    for p in range(P)
                ]
                mlp(seg, xa_d, qkv_a)
                mlp(seg, xb_d, qkv_b)
                attention(seg, 0, qkv_a[0], qkv_b[1], qkv_b[2])
                attention(seg, 1, qkv_b[0], qkv_a[1], qkv_a[2])

    nc.compile()
    return nc


def _preprocess(inputs):
    """Host-side folding + sharding. Returns (sched, in_maps, perm) where
    perm[core][pos] = original segment index handled at that position."""
    a = np.asarray(inputs["a"], dtype=np.float32)
    b = np.asarray(inputs["b"], dtype=np.float32)
    W1 = np.asarray(inputs["W1"], dtype=np.float32)
    b1 = np.asarray(inputs["b1"], dtype=np.float32)
    g = np.asarray(inputs["g"], dtype=np.float32)
    bt = np.asarray(inputs["bt"], dtype=np.float32)
    rm = np.asarray(inputs["rm"], dtype=np.float32)
    rv = np.asarray(inputs["rv"], dtype=np.float32)
    W2 = np.asarray(inputs["W2"], dtype=np.float32)
    b2 = np.asarray(inputs["b2"], dtype=np.float32)
    len_a = np.asarray(inputs["len_a"], dtype=np.int64)
    len_b = np.asarray(inputs["len_b"], dtype=np.int64)

    alpha = g / np.sqrt(rv + BN_EPS)
    beta = bt - rm * alpha
    W2p = W2 * alpha[:, :, None]
    b2p = b2 + np.einsum("ph,phd->pd", beta, W2)
    W2p[0] /= SCALE  # fold 1/32 score scale into the q net
    b2p[0] /= SCALE

    bf16 = ml_dtypes.bfloat16
    w1_bf = np.ascontiguousarray(W1.astype(bf16))
    w2_bf = np.ascontiguousarray(W2p.astype(bf16))

    # Segment -> (core, position) assignment. With RAGGED, sort by score cost
    # so each position's cross-core max (which fixes the SPMD loop bounds) is
    # as small as possible.
    if RAGGED:
        order = np.argsort(-(len_a * len_b), kind="stable")
    else:
        order = np.arange(B)
    perm = [[int(order[pos * N_CORES + c]) for pos in range(SEG)] for c in range(N_CORES)]

    # per-position structure = max over cores at that position
    sched = {}
    for pos in range(SEG):
        segs = [perm[c][pos] for c in range(N_CORES)]
        for dirn in range(2):
            lq = max((len_a if dirn == 0 else len_b)[s] for s in segs)
            lk = max((len_b if dirn == 0 else len_a)[s] for s in segs)
            if not RAGGED:
                lq, lk = LA, LB
            sched[(dirn, pos)] = (
                _round_up(int(lq), 128) // 128,
                _round_up(int(lk), 128),
            )

    iota = np.arange(LA)
    in_maps = []
    for c in range(N_CORES):
        segs = perm[c]
        xa = np.ascontiguousarray(a[segs].reshape(SEG * LA, D).astype(bf16))
        xb = np.ascontiguousarray(b[segs].reshape(SEG * LB, D).astype(bf16))
        km = np.zeros((2, SEG, LA), dtype=np.float32)
        wb = np.zeros((2, SEG, LA), dtype=np.float32)
        for pos, s in enumerate(segs):
            for dirn in range(2):
                lq = int((len_a if dirn == 0 else len_b)[s])
                lk = int((len_b if dirn == 0 else len_a)[s])
                km[dirn, pos, :] = np.where(iota < lk, 0.0, NEG)
                wb[dirn, pos, :] = np.where(iota < lq, 1.0 / lq, 0.0)
        in_maps.append(
            {
                "xa": xa,
                "xb": xb,
                "w1": w1_bf,
                "w2": w2_bf,
                "b1": np.ascontiguousarray(b1),
                "b2": np.ascontiguousarray(b2p),
                "km": np.ascontiguousarray(km.astype(bf16)),
                "wb": np.ascontiguousarray(wb),
            }
        )
    return sched, in_maps, perm


def kernel(**inputs):
    global LAST_RESULTS
    from concourse.bass_utils import run_bass_kernel_spmd

    sched, in_maps, perm = _preprocess(inputs)
    key = tuple(sorted(sched.items()))
    if key not in _CACHE:
        _CACHE[key] = _build_program(sched)
    nc = _CACHE[key]

    res = run_bass_kernel_spmd(nc, in_maps, list(range(N_CORES)))
    LAST_RESULTS = res

    out = np.zeros((2, B, D), dtype=np.float32)
    for c in range(N_CORES):
        o = res.results[c]["o"]  # [2, SEG, D]
        for pos, s in enumerate(perm[c]):
            out[0, s] = o[0, pos]
            out[1, s] = o[1, pos]
    return out
